# revision 4
# baseline (speedup 1.0000x reference)
"""Trainium2 Bass kernel for the GroupNorm->QKV->MHA->proj residual attention block.

Problem shapes (hardcoded): x [4, 128, 64, 64] f32, HEADS=4, GROUPS=32, L=4096.

Sharding: 16 (batch, head) pairs over 8 cores -> each core handles one batch and
two heads.  Each core computes GN + its heads' qkv + attention + a partial
projection over its 64 attention channels (+ 0.5*(x + b_proj)); the host sums
the two partials of each batch.
"""

import functools
import sys

sys.path.insert(0, "/opt/trn_rl_repo")

import numpy as np
import ml_dtypes

import concourse.bass as bass
import concourse.bacc as bacc
import concourse.tile as tile
from concourse import mybir
from concourse.bass_utils import run_bass_kernel_spmd

F32 = mybir.dt.float32
BF16 = mybir.dt.bfloat16

B, C, H, W = 4, 128, 64, 64
HEADS = 4
GROUPS = 32
EPS = 1e-5
L = H * W          # 4096
CH = C // HEADS    # 32
NCORES = 8
NCHUNK = L // 512  # 8 column chunks of 512
NST = L // 128     # 32 s-tiles of 128


def _bcast_ap(src, parts):
    """Partition-broadcast access pattern: read a [1, N] slice `parts` times."""
    return bass.AP(
        tensor=src.tensor,
        offset=src.offset,
        ap=[[0, parts]] + [list(d) for d in src.ap[1:]],
    )


def _body(tc, x, wqk, wv, bqk, bv, wp, hb, gmat, rcp_d, part):
    nc = tc.nc
    AF = mybir.ActivationFunctionType
    ALU = mybir.AluOpType

    from contextlib import ExitStack

    with ExitStack() as ctx:
        const = ctx.enter_context(tc.tile_pool(name="const", bufs=1))
        big = ctx.enter_context(tc.tile_pool(name="big", bufs=1))
        pbuf = ctx.enter_context(tc.tile_pool(name="pbuf", bufs=18))
        small = ctx.enter_context(tc.tile_pool(name="small", bufs=4))
        spsum = ctx.enter_context(tc.tile_pool(name="spsum", bufs=2, space="PSUM"))
        apsum = ctx.enter_context(tc.tile_pool(name="apsum", bufs=2, space="PSUM"))
        mpsum = ctx.enter_context(tc.tile_pool(name="mpsum", bufs=2, space="PSUM"))

        # ---- constants into SBUF ----
        wqk_sb = const.tile([C, 256], F32, tag="wqk")
        nc.sync.dma_start(out=wqk_sb, in_=wqk)
        wv_sb = const.tile([C, 2 * CH], F32, tag="wv")
        nc.sync.dma_start(out=wv_sb, in_=wv)
        bqk_sb = const.tile([C, 2], F32, tag="bqk")
        nc.sync.dma_start(out=bqk_sb, in_=bqk)
        bvb_sb = const.tile([C, 2 * CH], F32, tag="bvb")
        nc.sync.dma_start(out=bvb_sb, in_=_bcast_ap(bv, C))
        wps_sb = const.tile([C, 256], BF16, tag="wps")
        # proj weights for (head, col-group) at partitions 0-31 and 64-95
        nc.sync.dma_start(out=wps_sb[0:CH, 0:C], in_=wp[0:CH, :])
        nc.sync.dma_start(out=wps_sb[0:CH, C : 2 * C], in_=wp[CH : 2 * CH, :])
        nc.sync.dma_start(out=wps_sb[64 : 64 + CH, 0:C], in_=wp[0:CH, :])
        nc.sync.dma_start(out=wps_sb[64 : 64 + CH, C : 2 * C], in_=wp[CH : 2 * CH, :])
        hb_sb = const.tile([C, 1], F32, tag="hb")
        nc.sync.dma_start(out=hb_sb, in_=hb)
        gmat_sb = const.tile([C, C], F32, tag="gmat")
        nc.sync.dma_start(out=gmat_sb, in_=gmat)

        x_sb = big.tile([C, L], F32, tag="x")
        nc.sync.dma_start(out=x_sb, in_=x)

        # ---- GroupNorm statistics ----
        stats = small.tile([C, NCHUNK, 6], F32, tag="stats")
        for i in range(NCHUNK):
            nc.vector.bn_stats(out=stats[:, i, :], in_=x_sb[:, 512 * i : 512 * (i + 1)])
        mv = small.tile([C, 2], F32, tag="mv")
        nc.vector.bn_aggr(out=mv, in_=stats)
        # ms = [mean, var + mean^2] per channel
        ms = small.tile([C, 2], F32, tag="ms")
        nc.vector.tensor_copy(out=ms[:, 0:1], in_=mv[:, 0:1])
        sq = small.tile([C, 1], F32, tag="sq")
        nc.vector.tensor_mul(sq, mv[:, 0:1], mv[:, 0:1])
        nc.vector.tensor_add(ms[:, 1:2], mv[:, 1:2], sq)
        # group-average + broadcast via 0.25-blocked matmul
        gps = mpsum.tile([C, 2], F32, tag="mp")
        nc.tensor.matmul(gps, lhsT=gmat_sb, rhs=ms, start=True, stop=True)
        gsb = small.tile([C, 2], F32, tag="gsb")
        nc.vector.tensor_copy(out=gsb, in_=gps)
        gv = small.tile([C, 1], F32, tag="gv")
        nc.vector.tensor_mul(gv, gsb[:, 0:1], gsb[:, 0:1])
        nc.vector.tensor_sub(gv, gsb[:, 1:2], gv)  # gvar = E[x^2]_g - mean_g^2
        rstd = small.tile([C, 1], F32, tag="rstd")
        epst = small.tile([C, 1], F32, tag="epst")
        nc.vector.memset(epst, EPS)
        nc.scalar.activation(out=rstd, in_=gv, func=AF.Ln, bias=epst)
        nc.scalar.activation(out=rstd, in_=rstd, func=AF.Exp, scale=-0.5)
        hn = big.tile([C, L], F32, tag="hn")
        nc.vector.tensor_scalar(
            out=hn,
            in0=x_sb,
            scalar1=gsb[:, 0:1],
            scalar2=rstd,
            op0=ALU.subtract,
            op1=ALU.mult,
        )

        # ---- v^T tiles (both heads) with ones columns for the softmax rowsum ----
        # layout per l-tile i: [v_h0 (0:32) | 1 (32) | v_h1 (33:65) | 1 (65)]
        vt_all = big.tile([C, NST * 66], BF16, tag="vt")
        nc.vector.memset(vt_all, 1.0)
        for i in range(NST):
            pv = mpsum.tile([C, 2 * CH], F32, tag="mp")
            nc.tensor.matmul(
                pv, lhsT=hn[:, 128 * i : 128 * (i + 1)], rhs=wv_sb, start=True, stop=True
            )
            sl = vt_all[:, 66 * i : 66 * (i + 1)]
            nc.vector.tensor_add(sl[:, 0:CH], pv[:, 0:CH], bvb_sb[:, 0:CH])
            nc.vector.tensor_add(sl[:, 33 : 33 + CH], pv[:, CH : 2 * CH], bvb_sb[:, CH : 2 * CH])

        # ---- q/k, replicated per quadrant: rows 32*(2h+r) hold head h copy r ----
        # qkrep cols: [q (0:4096) | k (4096:8192)], bf16
        qkrep = big.tile([C, 2 * L], BF16, tag="qkrep")
        for half in range(2):  # 0 = q, 1 = k
            for cc in range(NCHUNK):
                pq = mpsum.tile([C, 512], F32, tag="mp")
                nc.tensor.matmul(
                    pq,
                    lhsT=wqk_sb[:, 128 * half : 128 * (half + 1)],
                    rhs=hn[:, 512 * cc : 512 * (cc + 1)],
                    start=True,
                    stop=True,
                )
                nc.vector.tensor_scalar_add(
                    out=qkrep[:, L * half + 512 * cc : L * half + 512 * (cc + 1)],
                    in0=pq,
                    scalar1=bqk_sb[:, half : half + 1],
                )

        # ---- attention (2 heads x 4 chunk-pairs) ----
        a_acc = big.tile([C, 2 * L // 2], BF16, tag="aacc")  # [128, 4096]
        for h in range(2):
            for p in range(4):
                aps = apsum.tile([C, 512], F32, tag="ap")
                for sh in range(2):  # halves of the s range
                    ptiles = {}
                    for g in range(8):
                        i0 = 16 * sh + 2 * g
                        for jc in range(2):  # chunk j = 2p + jc
                            j = 2 * p + jc
                            ps = spsum.tile([C, 1024], F32, tag="sp")
                            for r in range(2):
                                i = i0 + r
                                q0 = 32 * (2 * h + r)
                                nc.tensor.matmul(
                                    ps[:, 512 * r : 512 * (r + 1)],
                                    lhsT=qkrep[q0 : q0 + 32, L + 128 * i : L + 128 * (i + 1)],
                                    rhs=qkrep[q0 : q0 + 32, 512 * j : 512 * (j + 1)],
                                    start=True,
                                    stop=True,
                                    tile_position=(q0, 0),
                                )
                            pt = pbuf.tile([C, 1024], BF16, tag="p")
                            nc.scalar.activation(out=pt, in_=ps, func=AF.Exp)
                            ptiles[(g, jc)] = pt
                    for g in range(8):
                        i0 = 16 * sh + 2 * g
                        for r in range(2):
                            i = i0 + r
                            for jc in range(2):
                                nc.tensor.matmul(
                                    aps[64 * jc : 64 * jc + 33, :],
                                    lhsT=vt_all[:, 66 * i + 33 * h : 66 * i + 33 * h + 33],
                                    rhs=ptiles[(g, jc)][:, 512 * r : 512 * (r + 1)],
                                    start=(i == 0),
                                    stop=(i == NST - 1),
                                    skip_group_check=True,
                                )
                # normalize: a = A_raw * (1/rowsum); rowsum sits at partitions 32/96
                rcp = small.tile([C, 512], F32, tag="rcp")
                nc.vector.reciprocal(out=rcp[32:33, :], in_=aps[32:33, :])
                nc.vector.reciprocal(out=rcp[96:97, :], in_=aps[96:97, :])
                row = 2 * (4 * h + p)
                nc.sync.dma_start(out=rcp_d[row : row + 1, :], in_=rcp[32:33, :])
                nc.sync.dma_start(out=rcp_d[row + 1 : row + 2, :], in_=rcp[96:97, :])
                rcpb = small.tile([C, 512], F32, tag="rcpb")
                nc.sync.dma_start(out=rcpb[0:32, :], in_=_bcast_ap(rcp_d[row : row + 1, :], 32))
                nc.sync.dma_start(out=rcpb[64:96, :], in_=_bcast_ap(rcp_d[row + 1 : row + 2, :], 32))
                col = 512 * (4 * h + p)
                nc.vector.tensor_mul(a_acc[0:32, col : col + 512], aps[0:32, :], rcpb[0:32, :])
                nc.vector.tensor_mul(a_acc[64:96, col : col + 512], aps[64:96, :], rcpb[64:96, :])

        # ---- partial projection + 0.5 * (x + b_proj) ----
        for j in range(NCHUNK):
            p_, jc = j // 2, j % 2
            pp = mpsum.tile([C, 512], F32, tag="mp")
            for h in range(2):
                q0 = 64 * jc
                nc.tensor.matmul(
                    pp,
                    lhsT=wps_sb[q0 : q0 + 32, C * h : C * (h + 1)],
                    rhs=a_acc[q0 : q0 + 32, 512 * (4 * h + p_) : 512 * (4 * h + p_) + 512],
                    start=(h == 0),
                    stop=(h == 1),
                    tile_position=(q0, 0),
                )
            res = small.tile([C, 512], F32, tag="res")
            nc.vector.tensor_scalar(
                out=res,
                in0=x_sb[:, 512 * j : 512 * (j + 1)],
                scalar1=0.5,
                scalar2=hb_sb[:, 0:1],
                op0=ALU.mult,
                op1=ALU.add,
            )
            outt = small.tile([C, 512], F32, tag="outt")
            nc.vector.tensor_add(outt, pp, res)
            nc.sync.dma_start(out=part[:, 512 * j : 512 * (j + 1)], in_=outt)


@functools.lru_cache(maxsize=1)
def _build_program():
    nc = bacc.Bacc("TRN2", target_bir_lowering=False, debug=False, num_devices=NCORES)
    x = nc.dram_tensor("x", [C, L], F32, kind="ExternalInput").ap()
    wqk = nc.dram_tensor("wqk", [C, 256], F32, kind="ExternalInput").ap()
    wv = nc.dram_tensor("wv", [C, 2 * CH], F32, kind="ExternalInput").ap()
    bqk = nc.dram_tensor("bqk", [C, 2], F32, kind="ExternalInput").ap()
    bv = nc.dram_tensor("bv", [1, 2 * CH], F32, kind="ExternalInput").ap()
    wp = nc.dram_tensor("wp", [2 * CH, C], BF16, kind="ExternalInput").ap()
    hb = nc.dram_tensor("hb", [C, 1], F32, kind="ExternalInput").ap()
    gmat = nc.dram_tensor("gmat", [C, C], F32, kind="ExternalInput").ap()
    rcp_d = nc.dram_tensor("rcp_d", [16, 512], F32).ap()
    part = nc.dram_tensor("part", [C, L], F32, kind="ExternalOutput").ap()
    with tile.TileContext(nc) as tc:
        _body(tc, x, wqk, wv, bqk, bv, wp, hb, gmat, rcp_d, part)
    nc.compile()
    return nc


def make_in_maps(inputs):
    x = np.ascontiguousarray(np.asarray(inputs["x"], np.float32))
    gamma = np.asarray(inputs["gn_gamma"], np.float32)
    beta = np.asarray(inputs["gn_beta"], np.float32)
    w_qkv = np.asarray(inputs["w_qkv"], np.float32)
    b_qkv = np.asarray(inputs["b_qkv"], np.float32)
    w_proj = np.asarray(inputs["w_proj"], np.float32)
    b_proj = np.asarray(inputs["b_proj"], np.float32)

    scale = (1.0 / np.sqrt(np.sqrt(CH))).astype(np.float32)
    Wg = w_qkv * gamma[None, :]                  # fold GN gamma
    bf = b_qkv + w_qkv @ beta                    # fold GN beta
    gmat_np = np.zeros((C, C), np.float32)
    for g in range(GROUPS):
        gmat_np[g * 4 : (g + 1) * 4, g * 4 : (g + 1) * 4] = 0.25

    in_maps = []
    for core in range(NCORES):
        b = core // 2
        pi = core % 2
        hg = [2 * pi, 2 * pi + 1]  # global head ids of local heads 0, 1

        # wqk: cols [0:128] = q lhsT (rows [h0,h0,h1,h1] replicated per quadrant),
        #      cols [128:256] = k lhsT
        wqk_np = np.zeros((C, 256), np.float32)
        bqk_np = np.zeros((C, 2), np.float32)
        for lh, g in enumerate(hg):
            qW = Wg[CH * g : CH * (g + 1)] * scale          # [32, 128]
            kW = Wg[C + CH * g : C + CH * (g + 1)] * scale
            qb = bf[CH * g : CH * (g + 1)] * scale
            kb = bf[C + CH * g : C + CH * (g + 1)] * scale
            for r in range(2):
                m0 = 64 * lh + 32 * r
                wqk_np[:, m0 : m0 + 32] = qW.T
                wqk_np[:, 128 + m0 : 128 + m0 + 32] = kW.T
                bqk_np[m0 : m0 + 32, 0] = qb
                bqk_np[m0 : m0 + 32, 1] = kb

        wv_np = np.zeros((C, 2 * CH), np.float32)
        bv_np = np.zeros((1, 2 * CH), np.float32)
        for lh, g in enumerate(hg):
            wv_np[:, CH * lh : CH * (lh + 1)] = Wg[2 * C + CH * g : 2 * C + CH * (g + 1)].T
            bv_np[0, CH * lh : CH * (lh + 1)] = bf[2 * C + CH * g : 2 * C + CH * (g + 1)]

        wp_np = w_proj[:, 64 * pi : 64 * (pi + 1)].T.astype(ml_dtypes.bfloat16)  # [64, 128]
        hb_np = (0.5 * b_proj).reshape(C, 1).astype(np.float32)

        in_maps.append(
            {
                "x": x[b].reshape(C, L),
                "wqk": wqk_np,
                "wv": wv_np,
                "bqk": bqk_np,
                "bv": bv_np,
                "wp": np.ascontiguousarray(wp_np),
                "hb": hb_np,
                "gmat": gmat_np,
            }
        )
    return in_maps


def combine_outputs(results):
    out = np.empty((B, C, H, W), np.float32)
    for b in range(B):
        s = results[2 * b]["part"] + results[2 * b + 1]["part"]
        out[b] = s.reshape(C, H, W)
    return out


def _ensure_ntff_hook():
    """Register the axon NTFF profile hook if the environment lacks antenv.axon_hooks."""
    import types, contextlib, ctypes, os

    try:
        import antenv.axon_hooks  # noqa: F401
        return
    except ImportError:
        pass
    mod = types.ModuleType("antenv.axon_hooks")
    state = {"hook": None}
    mod.set_axon_ntff_profile_hook = lambda h: state.__setitem__("hook", h)
    mod.get_axon_ntff_profile_hook = lambda: state["hook"]
    sys.modules["antenv.axon_hooks"] = mod

    so_path = "/opt/axon/libaxon_pjrt.so"
    if not os.path.exists(so_path):
        return
    lib = ctypes.CDLL(so_path)
    if not hasattr(lib, "axon_start_nrt_profile"):
        return
    lib.axon_start_nrt_profile.argtypes = [ctypes.POINTER(ctypes.c_int64), ctypes.c_size_t]
    lib.axon_start_nrt_profile.restype = ctypes.c_int64
    lib.axon_stop_nrt_profile.argtypes = [ctypes.c_char_p]
    lib.axon_stop_nrt_profile.restype = ctypes.c_int64

    @contextlib.contextmanager
    def _hook(output_dir, device_ids):
        import jax

        jax.devices()
        if device_ids:
            ids = (ctypes.c_int64 * len(device_ids))(*device_ids)
            rc = lib.axon_start_nrt_profile(ids, len(device_ids))
        else:
            rc = lib.axon_start_nrt_profile(None, 0)
        if rc != 0:
            raise RuntimeError(f"axon_start_nrt_profile rc={rc}")
        try:
            yield
        finally:
            n = lib.axon_stop_nrt_profile(str(output_dir).encode())
            print(f"profile: {n} file(s) written to {output_dir}", file=sys.stderr)

    state["hook"] = _hook


def kernel_run(inputs, trace=False):
    nc = _build_program()
    in_maps = make_in_maps(inputs)
    if trace:
        _ensure_ntff_hook()
    res = run_bass_kernel_spmd(nc, in_maps, core_ids=list(range(NCORES)), trace=trace)
    return combine_outputs(res.results), res


def kernel(**inputs) -> np.ndarray:
    out, _ = kernel_run(inputs)
    return out
